# revision 1
# baseline (speedup 1.0000x reference)
"""GCN layer (gnn_message_passing) on 8 Trainium2 NeuronCores.

Math (matches torch_geometric GCNConv defaults / the jax reference):
    deg[d]  = sum_{e: dst=d} w_e + 1                      (self loop w=1)
    dinv    = deg^-1/2
    h       = x @ W
    out[d]  = relu( dinv[d] * ( sum_{e->d} w_e * dinv[src_e] * h[src_e]
                                + dinv[d] * h[d] )  + b )

Distribution: nodes sharded round-robin-contiguously across 8 cores
(6250/core, padded to 6272 = 49*128); edges partitioned by dst owner.

Per core program (SPMD, one compiled NEFF):
  1. deg: host lays each local node's in-edge weights (+1.0 self) into a
     padded row table -> one DVE reduce per 128-node tile; dinv via
     reciprocal+sqrt.
  2. h = x@W for the local shard (PE, f32), scaled to hs = dinv*h (bf16)
     and written to DRAM; hsb = dinv^2*h + b kept in SBUF (f32).
  3. AllGather hs -> full [50176,128] bf16 gather table in DRAM.
  4. Main pass over this core's edges in blocks of 128 (sorted by dst
     tile, sub-sorted by src half so int16 gather indices fit):
       - dma_gather 128-256 edge-rows/desc from the hs table (SWDGE)
       - one-hot S[e, dst_local] = w_e built in ONE DVE tensor_scalar
         (iota == dloc) * w
       - PE matmul  agg[dst,f] += S^T @ hs_gathered  accumulated in PSUM
         per dst tile
       - epilogue per tile: relu(dinv*agg + hsb) -> out rows.
Block structure is padded to the max over cores so all 8 cores run the
same program (dummy edges have w=0 -> contribute exactly 0).
"""

import math
import os
import sys

import numpy as np

P = 128           # partition / tile size
NCORES = 8
G_TILES = 4       # dst tiles per PSUM group
MAXBLK = 16       # max 128-edge blocks per dma_gather call
SINGLE_PACKET = False

_CACHE = {}


def _import_concourse():
    try:
        import concourse.bass  # noqa: F401
        return
    except ImportError:
        pass
    for p in ("/opt/trn_rl_repo", "/root/.axon_site/_ro/trn_rl_repo"):
        if os.path.isdir(p) and p not in sys.path:
            sys.path.insert(0, p)
    import concourse.bass  # noqa: F401


def _ceil(a, b):
    return -(-a // b)


def _preprocess(x, edge_index, edge_weight, W, b):
    """Shard + reorganize inputs on host. Returns (cfg, in_maps)."""
    x = np.asarray(x, dtype=np.float32)
    W = np.asarray(W, dtype=np.float32)
    b = np.asarray(b, dtype=np.float32)
    ei = np.asarray(edge_index)
    ew = np.asarray(edge_weight, dtype=np.float32)

    N, C = x.shape
    F = W.shape[1]
    assert C % P == 0 and F == P
    CH = C // P
    PER = _ceil(N, NCORES)
    NT = _ceil(PER, P)
    NP_ = NT * P
    NG = NCORES * NP_
    HALF = NG // 2
    assert HALF <= 32768, "int16 gather index range exceeded"

    src = ei[0].astype(np.int64)
    dst = ei[1].astype(np.int64)

    # gather table is split into two AllGather outputs A/B (keeps each
    # collective < 1MB/rank — the >1MB RDH algorithm hangs on this pod —
    # and keeps gather indices within int16). Node (r, l) lives in table
    # T = l // (NP/2) at row r*(NP/2) + l % (NP/2).
    HNP = NP_ // 2
    o_src = src // PER
    lsrc = src - o_src * PER
    half = lsrc // HNP                          # which table (0/1)
    gsrc = o_src * HNP + (lsrc % HNP)           # row within table
    owner = dst // PER
    ldst = dst - owner * PER
    tile_g = ldst // P
    dloc = ldst % P

    # per-core counts per (tile, half) -> unified block structure
    cnt = np.zeros((NCORES, NT, 2), np.int64)
    np.add.at(cnt, (owner, tile_g, half), 1)
    nb = _ceil(cnt, P).max(axis=0)              # [NT, 2]
    for t in range(NT):
        if nb[t].sum() == 0:
            nb[t][0] = 1

    # block layout: groups of G_TILES tiles; within group h0 run then h1 run
    blocks = []                                  # (tile, half)
    calls = []                                   # (half, b0, nblk)
    base = np.zeros((NT, 2), np.int64)
    for g0 in range(0, NT, G_TILES):
        tiles = range(g0, min(g0 + G_TILES, NT))
        for h in (0, 1):
            run_start = len(blocks)
            for t in tiles:
                base[t, h] = len(blocks)
                blocks.extend([(t, h)] * int(nb[t, h]))
            i = run_start
            while i < len(blocks):
                n = min(MAXBLK, len(blocks) - i)
                calls.append((h, i, n))
                i += n
    NBLK = len(blocks)
    NIDX = NBLK * P
    tile_first = {}
    tile_last = {}
    for i, (t, h) in enumerate(blocks):
        tile_first.setdefault(t, i)
        tile_last[t] = i

    # degree-table width (uniform across cores): max in-degree + 1 self
    deg_cnt = np.bincount(dst, minlength=N)
    PW = int(deg_cnt.max()) + 1

    import ml_dtypes
    in_maps = []
    iota = np.tile(np.arange(P, dtype=np.float32), (P, 1)).astype(
        ml_dtypes.bfloat16)
    B128 = np.tile(b[None, :], (P, 1)).astype(np.float32)

    for c in range(NCORES):
        m = owner == c
        s_c = gsrc[m]
        h_c = half[m]
        t_c = tile_g[m]
        dl_c = dloc[m]
        w_c = ew[m]
        ld_c = ldst[m]

        # ---- edge stream positions ------------------------------------
        key = t_c * 2 + h_c
        order = np.argsort(key, kind="stable")
        sk = key[order]
        grp_off = np.arange(len(sk)) - np.searchsorted(sk, sk)
        base_flat = base.reshape(-1)
        pos = base_flat[sk] * P + grp_off        # position in edge stream

        relidx = np.zeros(NIDX, np.int16)
        dlocf = np.zeros(NIDX, np.float32)
        wf = np.zeros(NIDX, np.float32)
        relidx[pos] = s_c[order].astype(np.int16)
        dlocf[pos] = dl_c[order].astype(np.float32)
        wf[pos] = w_c[order]

        idx16 = np.ascontiguousarray(
            np.tile(relidx.reshape(NIDX // 16, 16).T, (P // 16, 1)))
        dloc_arr = np.ascontiguousarray(dlocf.reshape(NBLK, P).T)
        wv_arr = np.ascontiguousarray(wf.reshape(NBLK, P).T)

        # ---- degree table [P, NT*PW]: row l%128, cols (l//128)*PW+j ---
        wpad = np.zeros((P, NT * PW), np.float32)
        o2 = np.argsort(ld_c, kind="stable")
        lds = ld_c[o2]
        ws = w_c[o2]
        off2 = np.arange(len(lds)) - np.searchsorted(lds, lds)
        wpad[lds % P, (lds // P) * PW + off2] = ws
        alln = np.arange(NP_)
        wpad[alln % P, (alln // P) * PW + np.minimum(
            np.bincount(ld_c, minlength=NP_), PW - 1)] = 1.0  # self loop

        # ---- xT shard [C, NP_] ----------------------------------------
        lo = c * PER
        hi = min((c + 1) * PER, N)
        xc = np.zeros((NP_, C), np.float32)
        xc[: hi - lo] = x[lo:hi]
        xT = np.ascontiguousarray(xc.T)

        in_maps.append({
            "xT": xT,
            "w_in": W,
            "bias128": B128,
            "iota": iota,
            "wpad": wpad,
            "dloc": dloc_arr,
            "wv": wv_arr,
            "idx16": idx16,
        })

    cfg = dict(N=N, C=C, F=F, CH=CH, PER=PER, NT=NT, NP=NP_, NG=NG,
               HALF=HALF, NBLK=NBLK, NIDX=NIDX, PW=PW,
               nb=tuple(map(tuple, nb.tolist())),
               blocks=tuple(blocks), calls=tuple(calls),
               tile_first=tuple(sorted(tile_first.items())),
               tile_last=tuple(sorted(tile_last.items())))
    return cfg, in_maps


def _build(cfg):
    _import_concourse()
    from concourse import bacc, mybir, tile
    dt = mybir.dt
    Alu = mybir.AluOpType
    Act = mybir.ActivationFunctionType
    X = mybir.AxisListType.X

    NT, NP_, NG = cfg["NT"], cfg["NP"], cfg["NG"]
    C, F, CH = cfg["C"], cfg["F"], cfg["CH"]
    HALF, NBLK, NIDX, PW = cfg["HALF"], cfg["NBLK"], cfg["NIDX"], cfg["PW"]
    blocks = cfg["blocks"]
    calls = cfg["calls"]
    tile_first = dict(cfg["tile_first"])
    tile_last = dict(cfg["tile_last"])

    nc = bacc.Bacc("TRN2", target_bir_lowering=False, debug=False,
                   num_devices=NCORES, num_swdge_queues=4)

    xT_d = nc.dram_tensor("xT", [C, NP_], dt.float32, kind="ExternalInput")
    W_d = nc.dram_tensor("w_in", [C, F], dt.float32, kind="ExternalInput")
    B_d = nc.dram_tensor("bias128", [P, F], dt.float32, kind="ExternalInput")
    iota_d = nc.dram_tensor("iota", [P, P], dt.bfloat16, kind="ExternalInput")
    wpad_d = nc.dram_tensor("wpad", [P, NT * PW], dt.float32,
                            kind="ExternalInput")
    dloc_d = nc.dram_tensor("dloc", [P, NBLK], dt.float32,
                            kind="ExternalInput")
    wv_d = nc.dram_tensor("wv", [P, NBLK], dt.float32, kind="ExternalInput")
    idx_d = nc.dram_tensor("idx16", [P, NIDX // 16], dt.int16,
                           kind="ExternalInput")
    out_d = nc.dram_tensor("out", [NP_, F], dt.float32, kind="ExternalOutput")
    hs_sh = nc.dram_tensor("hs_shard", [NP_, F], dt.bfloat16)
    HNP = NP_ // 2
    hs_ag = [
        nc.dram_tensor("hs_agA", [HALF, F], dt.bfloat16, addr_space="Shared"),
        nc.dram_tensor("hs_agB", [HALF, F], dt.bfloat16, addr_space="Shared"),
    ]
    # gather from plain-DRAM copies — SWDGE reads from Shared space are slow
    hs_tab = [
        nc.dram_tensor("hs_fullA", [HALF, F], dt.bfloat16),
        nc.dram_tensor("hs_fullB", [HALF, F], dt.bfloat16),
    ]

    with tile.TileContext(nc) as tc:
        with (
            tc.tile_pool(name="const", bufs=1) as cpool,
            tc.tile_pool(name="psum", bufs=8, space="PSUM") as ppool,
            tc.tile_pool(name="work", bufs=6) as wpool,
            tc.tile_pool(name="gather", bufs=12) as gpool,
            tc.tile_pool(name="sbuild", bufs=16) as spool,
            tc.tile_pool(name="xt", bufs=4) as xpool,
        ):
            # ---------------- const loads ------------------------------
            W_sb = []
            for ch in range(CH):
                t2 = cpool.tile([P, F], dt.float32, tag=f"W{ch}")
                nc.sync.dma_start(t2[:], W_d[ch * P:(ch + 1) * P, :])
                W_sb.append(t2)
            iota_sb = cpool.tile([P, P], dt.bfloat16, tag="iota")
            nc.sync.dma_start(iota_sb[:], iota_d[:])
            B_sb = cpool.tile([P, F], dt.float32, tag="B")
            nc.sync.dma_start(B_sb[:], B_d[:])
            wpad_sb = cpool.tile([P, NT * PW], dt.float32, tag="wpad")
            nc.sync.dma_start(wpad_sb[:], wpad_d[:])
            dloc_sb = cpool.tile([P, NBLK], dt.float32, tag="dloc")
            nc.sync.dma_start(dloc_sb[:], dloc_d[:])
            wv_sb = cpool.tile([P, NBLK], dt.float32, tag="wv")
            nc.sync.dma_start(wv_sb[:], wv_d[:])
            idx_sb = cpool.tile([P, NIDX // 16], dt.int16, tag="idx")
            nc.sync.dma_start(idx_sb[:], idx_d[:])
            hsb_loc = cpool.tile([P, NT * F], dt.float32, tag="hsb")
            dinv_sb = cpool.tile([P, NT], dt.float32, tag="dinv")
            dinv2_sb = cpool.tile([P, NT], dt.float32, tag="dinv2")

            # ---------------- phase 1: degrees -------------------------
            degt = wpool.tile([P, NT], dt.float32, tag="deg")
            for t in range(NT):
                nc.vector.reduce_sum(degt[:, t:t + 1],
                                     wpad_sb[:, t * PW:(t + 1) * PW], X)
            rec = wpool.tile([P, NT], dt.float32, tag="rec")
            nc.vector.reciprocal(rec[:], degt[:])
            nc.scalar.activation(dinv_sb[:], rec[:], Act.Sqrt)
            nc.vector.tensor_tensor(dinv2_sb[:], dinv_sb[:], dinv_sb[:],
                                    Alu.mult)

            # ---------------- phase 2: h = xW, hs, hsb -----------------
            for t in range(NT):
                ph = ppool.tile([P, F], dt.float32, tag="psum")
                for ch in range(CH):
                    xt_t = xpool.tile([P, P], dt.float32, tag="xt")
                    nc.sync.dma_start(
                        xt_t[:], xT_d[ch * P:(ch + 1) * P, t * P:(t + 1) * P])
                    nc.tensor.matmul(ph[:], xt_t[:],
                                     W_sb[ch][:], start=(ch == 0),
                                     stop=(ch == CH - 1))
                hsbf = wpool.tile([P, F], dt.bfloat16, tag="hsbf")
                nc.vector.tensor_scalar(hsbf[:], ph[:], dinv_sb[:, t:t + 1],
                                        None, Alu.mult)
                nc.sync.dma_start(hs_sh[t * P:(t + 1) * P, :], hsbf[:])
                nc.vector.scalar_tensor_tensor(
                    hsb_loc[:, t * F:(t + 1) * F], ph[:],
                    dinv2_sb[:, t:t + 1], B_sb[:], Alu.mult, Alu.add)

            # ---------------- phase 3: AllGather hs (2x <1MB) ----------
            for hh in (0, 1):
                nc.gpsimd.collective_compute(
                    "AllGather", Alu.bypass,
                    replica_groups=[list(range(NCORES))],
                    ins=[hs_sh[hh * HNP:(hh + 1) * HNP, :].opt()],
                    outs=[hs_ag[hh].ap().opt()],
                )
                nc.gpsimd.dma_start(hs_tab[hh].ap(), hs_ag[hh].ap())

            # ---------------- phase 4: gather + segment matmul ---------
            agg = {}
            for ci, (h, b0, nbc) in enumerate(calls):
                gb = gpool.tile([P, MAXBLK, F], dt.bfloat16, tag="gb")
                nc.gpsimd.dma_gather(
                    gb[:, :nbc, :],
                    hs_tab[h].ap(),
                    idx_sb[:, b0 * (P // 16):(b0 + nbc) * (P // 16)],
                    nbc * P, nbc * P, F, single_packet=SINGLE_PACKET,
                    queue_num=ci % 4)
                for j in range(nbc):
                    bi = b0 + j
                    t, _h = blocks[bi]
                    S = spool.tile([P, P], dt.bfloat16, tag="S")
                    nc.vector.tensor_scalar(S[:], iota_sb[:],
                                            dloc_sb[:, bi:bi + 1],
                                            wv_sb[:, bi:bi + 1],
                                            Alu.is_equal, Alu.mult)
                    if bi == tile_first[t]:
                        agg[t] = ppool.tile([P, F], dt.float32, tag="psum",
                                            name=f"agg{t}")
                    nc.tensor.matmul(agg[t][:], S[:], gb[:, j, :],
                                     start=(bi == tile_first[t]),
                                     stop=(bi == tile_last[t]))
                    if bi == tile_last[t]:
                        res = wpool.tile([P, F], dt.float32, tag="res")
                        nc.vector.scalar_tensor_tensor(
                            res[:], agg[t][:], dinv_sb[:, t:t + 1],
                            hsb_loc[:, t * F:(t + 1) * F],
                            Alu.mult, Alu.add)
                        ot = wpool.tile([P, F], dt.float32, tag="ot")
                        nc.scalar.activation(ot[:], res[:], Act.Relu)
                        nc.sync.dma_start(out_d[t * P:(t + 1) * P, :], ot[:])

    nc.compile()
    return nc


# knobs test.py can flip
TRACE = False
LAST_EXEC_NS = None
LAST_TRACE_PATH = None


def _cfg_key(cfg):
    return (cfg["N"], cfg["C"], cfg["F"], cfg["NBLK"], cfg["PW"],
            cfg["nb"], cfg["calls"])


def kernel(x, edge_index, edge_weight, W, b):
    global LAST_EXEC_NS, LAST_TRACE_PATH
    _import_concourse()
    from concourse import bass_utils

    cfg, in_maps = _preprocess(x, edge_index, edge_weight, W, b)
    key = _cfg_key(cfg)
    nc = _CACHE.get(key)
    if nc is None:
        nc = _build(cfg)
        _CACHE[key] = nc

    res = bass_utils.run_bass_kernel_spmd(
        nc, in_maps, core_ids=list(range(NCORES)), trace=TRACE)
    LAST_EXEC_NS = res.exec_time_ns
    if res.instructions_and_trace is not None:
        LAST_TRACE_PATH = res.instructions_and_trace[1]

    PER, N = cfg["PER"], cfg["N"]
    parts = []
    for c in range(NCORES):
        n_c = min(PER, N - c * PER)
        parts.append(res.results[c]["out"][:n_c])
    return np.ascontiguousarray(np.concatenate(parts, axis=0))



# revision 3
# speedup vs baseline: 1.7426x; 1.7426x over previous
"""GCN layer (gnn_message_passing) on 8 Trainium2 NeuronCores.

Math (matches torch_geometric GCNConv defaults / the jax reference):
    deg[d]  = sum_{e: dst=d} w_e + 1                      (self loop w=1)
    dinv    = deg^-1/2
    h       = x @ W
    out[d]  = relu( dinv[d] * ( sum_{e->d} w_e * dinv[src_e] * h[src_e]
                                + dinv[d] * h[d] )  + b )

Distribution: nodes sharded contiguously across 8 cores (6250/core,
padded to 6272 = 49*128); edges partitioned by dst owner.

Per core program (SPMD, one compiled NEFF):
  1. deg: host lays each local node's in-edge weights (+1.0 self) into a
     padded row table -> one DVE reduce per 128-node tile; dinv via
     reciprocal+sqrt.
  2. h = x@W for the local shard (PE, bf16 in / f32 psum), scaled to
     hs = dinv*h (bf16) and written to DRAM; hsb = dinv^2*h + b kept in
     SBUF (f32).
  3. AllGather hs -> full [50176,128] bf16 gather table, split in two
     halves A/B (keeps each collective < 1MB/rank - the >1MB RDH
     algorithm hangs on this pod - and keeps gather indices in int16
     range), copied from Shared to plain DRAM (SWDGE reads from Shared
     are slow).
  4. Main pass over this core's edges in blocks of 128 (sorted by dst
     tile, sub-sorted by src half so int16 gather indices fit):
       - dma_gather up to 16 blocks (2048 edge-rows) per SWDGE call
       - S[e, dst_local] = (iota==dloc)*w for the WHOLE call chunk in
         two DVE tensor_tensor ops using broadcast access patterns
       - PE matmul  agg[dst,f] += S_j^T @ hs_gathered_j  accumulated in
         PSUM per dst tile
       - epilogue per tile: relu(dinv*agg + hsb) -> out rows.
Block structure is padded to the max over cores so all 8 cores run the
same program (dummy edges have w=0 -> contribute exactly 0).
"""

import math
import os
import sys

import numpy as np

P = 128           # partition / tile size
NCORES = 8
G_TILES = 4       # dst tiles per PSUM group
MAXBLK = 16       # max 128-edge blocks per dma_gather call
SINGLE_PACKET = False

_CACHE = {}


def _import_concourse():
    try:
        import concourse.bass  # noqa: F401
        return
    except ImportError:
        pass
    for p in ("/opt/trn_rl_repo", "/root/.axon_site/_ro/trn_rl_repo"):
        if os.path.isdir(p) and p not in sys.path:
            sys.path.insert(0, p)
    import concourse.bass  # noqa: F401


def _ceil(a, b):
    return -(-a // b)


def _preprocess(x, edge_index, edge_weight, W, b):
    """Shard + reorganize inputs on host. Returns (cfg, in_maps)."""
    import ml_dtypes

    x = np.asarray(x, dtype=np.float32)
    W = np.asarray(W, dtype=np.float32)
    b = np.asarray(b, dtype=np.float32)
    ei = np.asarray(edge_index)
    ew = np.asarray(edge_weight, dtype=np.float32)

    N, C = x.shape
    F = W.shape[1]
    assert C % P == 0 and F == P
    CH = C // P
    PER = _ceil(N, NCORES)
    NT = _ceil(PER, P)
    NP_ = NT * P
    NG = NCORES * NP_
    HALF = NG // 2
    assert HALF <= 32768, "int16 gather index range exceeded"

    src = ei[0].astype(np.int64)
    dst = ei[1].astype(np.int64)

    # gather table split into two AllGather outputs A/B. Node (r, l)
    # lives in table T = l // (NP/2) at row r*(NP/2) + l % (NP/2).
    HNP = NP_ // 2
    o_src = src // PER
    lsrc = src - o_src * PER
    half = lsrc // HNP                          # which table (0/1)
    gsrc = o_src * HNP + (lsrc % HNP)           # row within table
    owner = dst // PER
    ldst = dst - owner * PER
    tile_g = ldst // P
    dloc = ldst % P

    # per-core counts per (tile, half) -> unified block structure
    cnt = np.zeros((NCORES, NT, 2), np.int64)
    np.add.at(cnt, (owner, tile_g, half), 1)
    nb = _ceil(cnt, P).max(axis=0)              # [NT, 2]
    for t in range(NT):
        if nb[t].sum() == 0:
            nb[t][0] = 1

    # block layout: groups of G_TILES tiles; within group h0 run then h1
    blocks = []                                  # (tile, half)
    calls = []                                   # (half, b0, nblk)
    base = np.zeros((NT, 2), np.int64)
    for g0 in range(0, NT, G_TILES):
        tiles = range(g0, min(g0 + G_TILES, NT))
        for h in (0, 1):
            run_start = len(blocks)
            for t in tiles:
                base[t, h] = len(blocks)
                blocks.extend([(t, h)] * int(nb[t, h]))
            i = run_start
            while i < len(blocks):
                n = min(MAXBLK, len(blocks) - i)
                calls.append((h, i, n))
                i += n
    NBLK = len(blocks)
    NIDX = NBLK * P
    tile_first = {}
    tile_last = {}
    for i, (t, h) in enumerate(blocks):
        tile_first.setdefault(t, i)
        tile_last[t] = i

    # degree-table width (uniform across cores): max in-degree + 1 self
    deg_cnt = np.bincount(dst, minlength=N)
    PW = int(deg_cnt.max()) + 1

    in_maps = []
    iota = np.tile(np.arange(P, dtype=np.float32), (P, 1)).astype(
        ml_dtypes.bfloat16)
    B128 = np.tile(b[None, :], (P, 1)).astype(np.float32)
    Wbf = W.astype(ml_dtypes.bfloat16)

    for c in range(NCORES):
        m = owner == c
        s_c = gsrc[m]
        h_c = half[m]
        t_c = tile_g[m]
        dl_c = dloc[m]
        w_c = ew[m]
        ld_c = ldst[m]

        # ---- edge stream positions ------------------------------------
        key = t_c * 2 + h_c
        order = np.argsort(key, kind="stable")
        sk = key[order]
        grp_off = np.arange(len(sk)) - np.searchsorted(sk, sk)
        base_flat = base.reshape(-1)
        pos = base_flat[sk] * P + grp_off        # position in edge stream

        relidx = np.zeros(NIDX, np.int16)
        dlocf = np.zeros(NIDX, np.float32)
        wf = np.zeros(NIDX, np.float32)
        relidx[pos] = s_c[order].astype(np.int16)
        dlocf[pos] = dl_c[order].astype(np.float32)
        wf[pos] = w_c[order]

        idx16 = np.ascontiguousarray(
            np.tile(relidx.reshape(NIDX // 16, 16).T, (P // 16, 1)))
        dloc_arr = np.ascontiguousarray(
            dlocf.reshape(NBLK, P).T).astype(ml_dtypes.bfloat16)
        wv_arr = np.ascontiguousarray(
            wf.reshape(NBLK, P).T).astype(ml_dtypes.bfloat16)

        # ---- degree table [P, NT*PW]: row l%128, cols (l//128)*PW+j ---
        wpad = np.zeros((P, NT * PW), np.float32)
        o2 = np.argsort(ld_c, kind="stable")
        lds = ld_c[o2]
        ws = w_c[o2]
        off2 = np.arange(len(lds)) - np.searchsorted(lds, lds)
        wpad[lds % P, (lds // P) * PW + off2] = ws
        alln = np.arange(NP_)
        wpad[alln % P, (alln // P) * PW + np.minimum(
            np.bincount(ld_c, minlength=NP_), PW - 1)] = 1.0  # self loop

        # ---- xT shard [C, NP_] bf16 -----------------------------------
        lo = c * PER
        hi = min((c + 1) * PER, N)
        xc = np.zeros((NP_, C), np.float32)
        xc[: hi - lo] = x[lo:hi]
        xT = np.ascontiguousarray(xc.T).astype(ml_dtypes.bfloat16)

        in_maps.append({
            "xT": xT,
            "w_in": Wbf,
            "bias128": B128,
            "iota": iota,
            "wpad": wpad,
            "dloc": dloc_arr,
            "wv": wv_arr,
            "idx16": idx16,
        })

    cfg = dict(N=N, C=C, F=F, CH=CH, PER=PER, NT=NT, NP=NP_, NG=NG,
               HALF=HALF, NBLK=NBLK, NIDX=NIDX, PW=PW,
               nb=tuple(map(tuple, nb.tolist())),
               blocks=tuple(blocks), calls=tuple(calls),
               tile_first=tuple(sorted(tile_first.items())),
               tile_last=tuple(sorted(tile_last.items())))
    return cfg, in_maps


def _build(cfg):
    _import_concourse()
    from concourse import bacc, mybir, tile
    from concourse.bass import AP
    dt = mybir.dt
    Alu = mybir.AluOpType
    Act = mybir.ActivationFunctionType
    X = mybir.AxisListType.X

    NT, NP_, NG = cfg["NT"], cfg["NP"], cfg["NG"]
    C, F, CH = cfg["C"], cfg["F"], cfg["CH"]
    HALF, NBLK, NIDX, PW = cfg["HALF"], cfg["NBLK"], cfg["NIDX"], cfg["PW"]
    blocks = cfg["blocks"]
    calls = cfg["calls"]
    tile_first = dict(cfg["tile_first"])
    tile_last = dict(cfg["tile_last"])

    nc = bacc.Bacc("TRN2", target_bir_lowering=False, debug=False,
                   num_devices=NCORES, num_swdge_queues=4)

    xT_d = nc.dram_tensor("xT", [C, NP_], dt.bfloat16, kind="ExternalInput")
    W_d = nc.dram_tensor("w_in", [C, F], dt.bfloat16, kind="ExternalInput")
    B_d = nc.dram_tensor("bias128", [P, F], dt.float32, kind="ExternalInput")
    iota_d = nc.dram_tensor("iota", [P, P], dt.bfloat16, kind="ExternalInput")
    wpad_d = nc.dram_tensor("wpad", [P, NT * PW], dt.float32,
                            kind="ExternalInput")
    dloc_d = nc.dram_tensor("dloc", [P, NBLK], dt.bfloat16,
                            kind="ExternalInput")
    wv_d = nc.dram_tensor("wv", [P, NBLK], dt.bfloat16, kind="ExternalInput")
    idx_d = nc.dram_tensor("idx16", [P, NIDX // 16], dt.int16,
                           kind="ExternalInput")
    out_d = nc.dram_tensor("out", [NP_, F], dt.float32, kind="ExternalOutput")
    hs_sh = nc.dram_tensor("hs_shard", [NP_, F], dt.bfloat16)
    HNP = NP_ // 2
    hs_ag = [
        nc.dram_tensor("hs_agA", [HALF, F], dt.bfloat16, addr_space="Shared"),
        nc.dram_tensor("hs_agB", [HALF, F], dt.bfloat16, addr_space="Shared"),
    ]
    # gather from plain-DRAM copies - SWDGE reads from Shared space are slow
    hs_tab = [
        nc.dram_tensor("hs_fullA", [HALF, F], dt.bfloat16),
        nc.dram_tensor("hs_fullB", [HALF, F], dt.bfloat16),
    ]

    with tile.TileContext(nc) as tc:
        with (
            tc.tile_pool(name="const", bufs=1) as cpool,
            tc.tile_pool(name="psum", bufs=8, space="PSUM") as ppool,
            tc.tile_pool(name="work", bufs=6) as wpool,
            tc.tile_pool(name="gather", bufs=12) as gpool,
            tc.tile_pool(name="sbuild", bufs=8) as spool,
            tc.tile_pool(name="xt", bufs=4) as xpool,
        ):
            # ---------------- const loads ------------------------------
            W_sb = []
            for ch in range(CH):
                t2 = cpool.tile([P, F], dt.bfloat16, tag=f"W{ch}")
                nc.sync.dma_start(t2[:], W_d[ch * P:(ch + 1) * P, :])
                W_sb.append(t2)
            iota_sb = cpool.tile([P, P], dt.bfloat16, tag="iota")
            nc.sync.dma_start(iota_sb[:], iota_d[:])
            B_sb = cpool.tile([P, F], dt.float32, tag="B")
            nc.sync.dma_start(B_sb[:], B_d[:])
            wpad_sb = cpool.tile([P, NT * PW], dt.float32, tag="wpad")
            nc.sync.dma_start(wpad_sb[:], wpad_d[:])
            dloc_sb = cpool.tile([P, NBLK], dt.bfloat16, tag="dloc")
            nc.sync.dma_start(dloc_sb[:], dloc_d[:])
            wv_sb = cpool.tile([P, NBLK], dt.bfloat16, tag="wv")
            nc.sync.dma_start(wv_sb[:], wv_d[:])
            idx_sb = cpool.tile([P, NIDX // 16], dt.int16, tag="idx")
            nc.sync.dma_start(idx_sb[:], idx_d[:])
            hsb_loc = cpool.tile([P, NT * F], dt.float32, tag="hsb")
            dinv_sb = cpool.tile([P, NT], dt.float32, tag="dinv")
            dinv2_sb = cpool.tile([P, NT], dt.float32, tag="dinv2")

            # ---------------- phase 1: degrees -------------------------
            degt = wpool.tile([P, NT], dt.float32, tag="deg")
            for t in range(NT):
                nc.vector.reduce_sum(degt[:, t:t + 1],
                                     wpad_sb[:, t * PW:(t + 1) * PW], X)
            rec = wpool.tile([P, NT], dt.float32, tag="rec")
            nc.vector.reciprocal(rec[:], degt[:])
            nc.scalar.activation(dinv_sb[:], rec[:], Act.Sqrt)
            nc.vector.tensor_tensor(dinv2_sb[:], dinv_sb[:], dinv_sb[:],
                                    Alu.mult)

            # ---------------- phase 2: h = xW, hs, hsb -----------------
            for t in range(NT):
                ph = ppool.tile([P, F], dt.float32, tag="psum")
                for ch in range(CH):
                    xt_t = xpool.tile([P, P], dt.bfloat16, tag="xt")
                    nc.sync.dma_start(
                        xt_t[:], xT_d[ch * P:(ch + 1) * P, t * P:(t + 1) * P])
                    nc.tensor.matmul(ph[:], xt_t[:],
                                     W_sb[ch][:], start=(ch == 0),
                                     stop=(ch == CH - 1))
                hsbf = wpool.tile([P, F], dt.bfloat16, tag="hsbf")
                nc.vector.tensor_scalar(hsbf[:], ph[:], dinv_sb[:, t:t + 1],
                                        None, Alu.mult)
                nc.sync.dma_start(hs_sh[t * P:(t + 1) * P, :], hsbf[:])
                nc.vector.scalar_tensor_tensor(
                    hsb_loc[:, t * F:(t + 1) * F], ph[:],
                    dinv2_sb[:, t:t + 1], B_sb[:], Alu.mult, Alu.add)

            # ---------------- phase 3: AllGather hs (2x <1MB) ----------
            for hh in (0, 1):
                nc.gpsimd.collective_compute(
                    "AllGather", Alu.bypass,
                    replica_groups=[list(range(NCORES))],
                    ins=[hs_sh[hh * HNP:(hh + 1) * HNP, :].opt()],
                    outs=[hs_ag[hh].ap().opt()],
                )
                nc.gpsimd.dma_start(hs_tab[hh].ap(), hs_ag[hh].ap())

            # ---------------- phase 4: gather + segment matmul ---------
            agg = {}
            for ci, (h, b0, nbc) in enumerate(calls):
                gb = gpool.tile([P, MAXBLK, F], dt.bfloat16, tag="gb")
                nc.gpsimd.dma_gather(
                    gb[:, :nbc, :],
                    hs_tab[h].ap(),
                    idx_sb[:, b0 * (P // 16):(b0 + nbc) * (P // 16)],
                    nbc * P, nbc * P, F, single_packet=SINGLE_PACKET,
                    queue_num=ci % 4)
                # S for the whole chunk in 2 DVE ops:
                #   eq[p, j, q] = (dloc[p, b0+j] == iota[p, q])
                #   S[p, j, q]  = eq * wv[p, b0+j]
                Sall = spool.tile([P, MAXBLK, P], dt.bfloat16, tag="S")
                eqt = spool.tile([P, MAXBLK, P], dt.bfloat16, tag="eq")
                dloc_bc = dloc_sb[:, b0:b0 + nbc].to_broadcast([P, nbc, P])
                wv_bc = wv_sb[:, b0:b0 + nbc].to_broadcast([P, nbc, P])
                iota_ap = iota_sb[:, :]
                iota_bc = AP(iota_ap.tensor, iota_ap.offset,
                             [iota_ap.ap[0], [0, nbc], iota_ap.ap[1]])
                nc.vector.tensor_tensor(eqt[:, :nbc, :], dloc_bc, iota_bc,
                                        Alu.is_equal)
                nc.vector.tensor_tensor(Sall[:, :nbc, :], eqt[:, :nbc, :],
                                        wv_bc, Alu.mult)
                for j in range(nbc):
                    bi = b0 + j
                    t, _h = blocks[bi]
                    if bi == tile_first[t]:
                        agg[t] = ppool.tile([P, F], dt.float32, tag="psum",
                                            name=f"agg{t}")
                    nc.tensor.matmul(agg[t][:], Sall[:, j, :], gb[:, j, :],
                                     start=(bi == tile_first[t]),
                                     stop=(bi == tile_last[t]))
                    if bi == tile_last[t]:
                        res = wpool.tile([P, F], dt.float32, tag="res")
                        nc.vector.scalar_tensor_tensor(
                            res[:], agg[t][:], dinv_sb[:, t:t + 1],
                            hsb_loc[:, t * F:(t + 1) * F],
                            Alu.mult, Alu.add)
                        ot = wpool.tile([P, F], dt.float32, tag="ot")
                        nc.scalar.activation(ot[:], res[:], Act.Relu)
                        nc.sync.dma_start(out_d[t * P:(t + 1) * P, :], ot[:])

    nc.compile()
    return nc


# knobs test.py can flip
TRACE = False
LAST_EXEC_NS = None
LAST_TRACE_PATH = None


def _cfg_key(cfg):
    return (cfg["N"], cfg["C"], cfg["F"], cfg["NBLK"], cfg["PW"],
            cfg["nb"], cfg["calls"])


def kernel(x, edge_index, edge_weight, W, b):
    global LAST_EXEC_NS, LAST_TRACE_PATH
    _import_concourse()
    from concourse import bass_utils

    cfg, in_maps = _preprocess(x, edge_index, edge_weight, W, b)
    key = _cfg_key(cfg)
    nc = _CACHE.get(key)
    if nc is None:
        nc = _build(cfg)
        _CACHE[key] = nc

    res = bass_utils.run_bass_kernel_spmd(
        nc, in_maps, core_ids=list(range(NCORES)), trace=TRACE)
    LAST_EXEC_NS = res.exec_time_ns
    if res.instructions_and_trace is not None:
        LAST_TRACE_PATH = res.instructions_and_trace[1]

    PER, N = cfg["PER"], cfg["N"]
    parts = []
    for c in range(NCORES):
        n_c = min(PER, N - c * PER)
        parts.append(res.results[c]["out"][:n_c])
    return np.ascontiguousarray(np.concatenate(parts, axis=0))
